# revision 23
# baseline (speedup 1.0000x reference)
"""Trainium2 Bass kernel for nn_CellGate (hetero GNN message passing + LSTM-style gate).

Strategy (8-core SPMD, dst-sharded), v2:
- Each core owns a contiguous 12,500-node shard of both node types (A and B).
- Segment-mean aggregation per edge type via `dma_gather` slot passes (as v1),
  but with small pass widths (<=24 groups), deep idx/msg pools and round-robin
  SWDGE queues so all four Q7 descriptor-generation pairs run concurrently.
- Per-(et,range) flush: one DVE cast f32->bf16 of the slot accumulator, then 4
  `dma_scatter_add`s (bf16, 256B rows) into a host-zeroed DRAM master.
- Dense stage is transpose-free: masters are read back with HWDGE DMA-transpose
  into feature-major bf16 [64, nodes] tiles, recip (mean) applied there once,
  and per-128-node-chunk matmuls consume the same feature-major tile as lhsT
  (node-major output) and as rhs (feature-major output for the next layer's
  W_r term). Biases ride in via partition-replicated adds / activation bias.
- Weights (incl. the Wx input-projection folds for layer 0) are folded on host
  and shipped bf16.
- One AllGather per node type rebuilds the full f32 gather table between
  layers; gates are elementwise on DVE at the end.
"""

import numpy as np

import concourse.bass as bass
import concourse.bacc as bacc
import concourse.mybir as mybir
import concourse.tile as tile

P = 128
D = 64

# edge types: (name, src_type, dst_type)
ETS = [("AB", 0, 1), ("BA", 1, 0), ("AA", 0, 0)]
L = 2

WCAP_G = 24              # max gather pass width in groups of 128
WSET_G = list(range(1, 26))   # width register values (multiples of 128)


def full_cfg():
    return dict(n_cores=8, shard=12500, G=98, n_ranges=4, spr=2)


def cfg_derived(cfg):
    c = dict(cfg)
    c["pad"] = P * c["G"]
    c["rwin"] = c["spr"] * c["pad"]
    c["trows"] = c["n_cores"] * c["pad"]
    c["nnodes"] = c["n_cores"] * c["shard"]
    c["wset"] = [g * P for g in WSET_G]
    # scatter chunk group ranges (4 chunks per flush)
    sch = []
    g0 = 0
    while g0 < c["G"]:
        g1 = min(g0 + 25, c["G"])
        sch.append((g0, g1))
        g0 = g1
    c["scatter_chunks"] = sch
    return c


def roundw(w, wset):
    for v in wset:
        if v >= w:
            return v
    return wset[-1]


# ---------------------------------------------------------------- host prep

def host_prep(cfg, edges):
    """edges: dict name -> [2, E] int32 (src, dst global).

    Returns: schedule (static, shared): list of passes (et_i, r, k, go, wp)
    and per-core arrays:
      gidx[core]: int16 [128, CBTOT]  (packed gather indices, 8-replicated)
      sidx[core]: int16 [3, n_ranges, 128, pad//16]
      deg[core]:  int32 [3, pad]   (total in-degree per et)
    """
    cfg = cfg_derived(cfg)
    NCO, SH, PAD, G = cfg["n_cores"], cfg["shard"], cfg["pad"], cfg["G"]
    NR, RWIN, WSET = cfg["n_ranges"], cfg["rwin"], cfg["wset"]

    percore = [dict(sidx=np.zeros((3, NR, 128, PAD // 16), np.int16),
                    deg=np.zeros((3, PAD), np.int32)) for _ in range(NCO)]

    all_counts = {}   # (et_i, r, k) -> max over cores of count
    maxk = {}         # (et_i, r) -> K
    core_data = {}    # (core, et_i, r) -> (theta, by_k list)
    for et_i, (etn, sT, dT) in enumerate(ETS):
        src, dst = edges[etn][0].astype(np.int64), edges[etn][1].astype(np.int64)
        srow = PAD * (src // SH) + (src - SH * (src // SH))  # global table row
        for c in range(NCO):
            m = (dst // SH) == c
            s_r, d_l = srow[m], dst[m] - c * SH
            percore[c]["deg"][et_i] = np.bincount(d_l, minlength=PAD)[:PAD]
            for r in range(NR):
                rm = (s_r // RWIN) == r
                sl, dl = s_r[rm] - r * RWIN, d_l[rm]
                deg_r = np.bincount(dl, minlength=PAD)[:PAD]
                theta = np.argsort(-deg_r, kind="stable")
                slot_of = np.empty(PAD, np.int64)
                slot_of[theta] = np.arange(PAD)
                K = int(deg_r.max()) if deg_r.size else 0
                maxk[(et_i, r)] = max(maxk.get((et_i, r), 0), K)
                order = np.argsort(dl, kind="stable")
                dls, sls = dl[order], sl[order]
                if dls.size:
                    starts = np.r_[0, np.nonzero(np.diff(dls))[0] + 1]
                    rank = np.arange(dls.size) - np.repeat(
                        starts, np.diff(np.r_[starts, dls.size]))
                else:
                    rank = np.zeros(0, np.int64)
                by_k = []
                for k in range(K):
                    km = rank == k
                    dk, sk = dls[km], sls[km]
                    all_counts[(et_i, r, k)] = max(
                        all_counts.get((et_i, r, k), 0), dk.size)
                    by_k.append((slot_of[dk], sk))
                core_data[(c, et_i, r)] = (theta, by_k)
                w = theta.astype(np.int16).reshape(PAD // 16, 16).T
                percore[c]["sidx"][et_i, r] = np.tile(w, (8, 1))

    # static schedule, pieces of <= WCAP_G groups, round-robin across ets
    schedule = []
    for et_i in range(3):
        for r in range(NR):
            for k in range(maxk.get((et_i, r), 0)):
                w = ((max(all_counts.get((et_i, r, k), 1), 1) + P - 1) // P) * P
                go = 0
                while go * P < w:
                    gw = min(WCAP_G, w // P - go)
                    wp = roundw(gw * P, WSET)
                    schedule.append((et_i, r, k, go, wp))
                    go += wp // P
    # stagger range order per et so flushes don't cluster across ets
    def rord(e, r):
        return (r - e) % NR
    per_et = [[] for _ in range(3)]
    for t in sorted(schedule, key=lambda t: (t[0], rord(t[0], t[1]), t[2], t[3])):
        per_et[t[0]].append(t)
    schedule = []
    i = [0, 0, 0]
    while any(i[e] < len(per_et[e]) for e in range(3)):
        for e in range(3):
            if i[e] < len(per_et[e]):
                schedule.append(per_et[e][i[e]])
                i[e] += 1

    # packed gather idx per core
    ZLOC = SH  # zero-row local index within each range window
    for c in range(NCO):
        full_arr = {}
        for (et_i, r, k, go, wp) in schedule:
            key = (c, et_i, r, k)
            if key not in full_arr:
                theta, by_k = core_data[(c, et_i, r)]
                arr = np.full(PAD, ZLOC, np.int16)
                if k < len(by_k):
                    slots, srcs = by_k[k]
                    arr[slots] = srcs.astype(np.int16)
                full_arr[key] = arr
        parts = []
        for (et_i, r, k, go, wp) in schedule:
            arr = np.full(wp, ZLOC, np.int16)
            seg = full_arr[(c, et_i, r, k)][go * P:go * P + wp]
            arr[:seg.size] = seg
            wrapped = arr.reshape(wp // 16, 16).T
            parts.append(np.tile(wrapped, (8, 1)))
        percore[c]["gidx"] = np.concatenate(parts, axis=1)
    cbtot = percore[0]["gidx"].shape[1]
    return cfg, schedule, percore, cbtot


# ---------------------------------------------------------------- builder

def build(cfg, schedule, cbtot, skip_gather=False, skip_dense=False,
          skip_cc=False, skip_scatter=False):
    cfg = cfg_derived(cfg)
    NCO, SH, PAD, G = cfg["n_cores"], cfg["shard"], cfg["pad"], cfg["G"]
    NR, RWIN, TROWS, WSET = cfg["n_ranges"], cfg["rwin"], cfg["trows"], cfg["wset"]
    f32 = mybir.dt.float32
    bf16 = mybir.dt.bfloat16
    i16 = mybir.dt.int16
    HG = 49                    # groups per dense half
    HR = HG * P                # rows per dense half
    BL = 4                     # dense groups per block
    GCHUNK = 20                # gates groups per chunk

    nc = bacc.Bacc(None, target_bir_lowering=False, debug=False,
                   num_swdge_queues=4, num_devices=NCO)

    # ---------------- inputs
    tabx = [nc.declare_dram_parameter(f"tabx{t}", [TROWS, D], f32, isOutput=False)
            for t in "AB"]
    xf = [nc.declare_dram_parameter(f"xf{t}", [D, PAD], bf16, isOutput=False)
          for t in "AB"]
    gates_in = {}
    for t in "AB":
        for nmm in "cif":
            gates_in[nmm + t] = nc.declare_dram_parameter(
                f"{nmm}{t}", [PAD, D], f32, isOutput=False)
    wnode = nc.declare_dram_parameter("wnode", [L, 3, D, D], bf16, isOutput=False)
    wrn = nc.declare_dram_parameter("wrn", [L, 2, D, D], bf16, isOutput=False)
    biasrep = nc.declare_dram_parameter("biasrep", [L, 2, P, D], f32, isOutput=False)
    blf = nc.declare_dram_parameter("blf", [D, 2], f32, isOutput=False)
    recipf = nc.declare_dram_parameter("recipf", [3, D, PAD], bf16, isOutput=False)
    masters = [[nc.declare_dram_parameter(f"m{e}_{l}", [PAD, P], bf16,
                                          isOutput=False)
                for e in range(3)] for l in range(L)]
    gidx = nc.declare_dram_parameter("gidx", [128, cbtot], i16, isOutput=False)
    sidx = nc.declare_dram_parameter("sidx", [3, NR, 128, PAD // 16], i16,
                                     isOutput=False)
    outs = [nc.declare_dram_parameter(f"out{t}", [PAD, D], f32, isOutput=True)
            for t in "AB"]

    # ---------------- DRAM internals
    stg = [nc.dram_tensor(f"stg{t}", [PAD, D], f32) for t in "AB"]
    tf1 = [nc.dram_tensor(f"tf1{t}", [D, PAD], bf16) for t in "AB"]
    tab_space = "Shared" if NCO > 4 else "Local"
    tab1 = [nc.dram_tensor(f"tab1{t}", [TROWS, D], f32, addr_space=tab_space)
            for t in "AB"]

    # ---------------- width registers (before TileContext)
    wregs = {}
    for w in WSET:
        r = nc.alloc_register(mybir.EngineType.Pool, f"w{w}")
        nc.gpsimd.reg_mov(r, w)
        wregs[w] = r

    rearr = "(p g) d -> p g d"

    with tile.TileContext(nc) as tc:
        with tc.tile_pool(name="const", bufs=1) as cpool, \
             tc.tile_pool(name="accp", bufs=2) as apool, \
             tc.tile_pool(name="idxp", bufs=16) as ipool, \
             tc.tile_pool(name="sidxp", bufs=2) as spool, \
             tc.tile_pool(name="msgp", bufs=7) as mpool, \
             tc.tile_pool(name="aggfp", bufs=1) as gpool, \
             tc.tile_pool(name="recfp", bufs=2) as rpool, \
             tc.tile_pool(name="densep", bufs=3) as dpool, \
             tc.tile_pool(name="psA", bufs=2, space="PSUM") as psA, \
             tc.tile_pool(name="psB", bufs=2, space="PSUM") as psB, \
             tc.tile_pool(name="psFA", bufs=2, space="PSUM") as psFA, \
             tc.tile_pool(name="psFB", bufs=2, space="PSUM") as psFB:

            # ---- constants
            wn_t = cpool.tile([D, L * 3, D], bf16)
            nc.sync.dma_start(out=wn_t[:], in_=wnode[:].rearrange("l e a b -> a (l e) b"))
            wr_t = cpool.tile([D, L * 2, D], bf16)
            nc.sync.dma_start(out=wr_t[:], in_=wrn[:].rearrange("l t a b -> a (l t) b"))
            brep_t = cpool.tile([P, L * 2, D], f32)
            nc.sync.dma_start(out=brep_t[:], in_=biasrep[:].rearrange("l t p d -> p (l t) d"))
            blf_t = cpool.tile([D, 2], f32)
            nc.sync.dma_start(out=blf_t[:], in_=blf[:])

            zero_small = cpool.tile([P, D], f32)
            nc.vector.memset(zero_small[:], 0.0)

            # ============ per layer ============
            for l in range(L):
                tabs = [tabx[0], tabx[1]] if l == 0 else [tab1[0], tab1[1]]
                last = (l == L - 1)

                accs = [apool.tile([P, G, D], bf16, tag=f"acc{e}", name=f"acc_{l}_{e}")
                        for e in range(3)]
                cur_r = [None, None, None]
                for (e, r, k, go, wp) in schedule:
                    if cur_r[e] is None:
                        cur_r[e] = r
                for e in range(3):
                    nc.vector.memset(accs[e][:], 0.0)

                def flush(e, r, accs=accs, l=l):
                    # scatter-add the bf16 slot accumulator into the master
                    sx = spool.tile([P, PAD // 16], i16, tag="sx",
                                    name=f"sx_{l}_{e}_{r}")
                    nc.sync.dma_start(out=sx[:], in_=sidx[e, r])
                    if skip_scatter:
                        return
                    for (g0, g1) in cfg["scatter_chunks"]:
                        w = (g1 - g0) * P
                        nc.gpsimd.dma_scatter_add(
                            masters[l][e][:, 0:D], accs[e][:, g0:g1, :],
                            sx[:, g0 * 8:g1 * 8], w, wregs[w], D,
                            elem_step=P,
                            single_packet=False, queue_num=0)

                col = 0
                qn = 0
                for (e, r, k, go, wp) in schedule:
                    if r != cur_r[e]:
                        flush(e, cur_r[e])
                        cur_r[e] = r
                        accs[e] = apool.tile([P, G, D], bf16, tag=f"acc{e}",
                                             name=f"acc_{l}_{e}_r{r}")
                        nc.vector.memset(accs[e][:], 0.0)
                    cb = wp // 16
                    gw = wp // P
                    idx_t = ipool.tile([P, cb], i16, tag="gi", name=f"gi_{l}_{qn}")
                    nc.sync.dma_start(out=idx_t[:], in_=gidx[:, col:col + cb])
                    msg = mpool.tile([P, gw, D], f32, tag="msg", name=f"msg_{l}_{qn}")
                    sT = ETS[e][1]
                    if not skip_gather:
                        nc.gpsimd.dma_gather(
                            out_ap=msg[:],
                            in_ap=tabs[sT][r * RWIN:(r + 1) * RWIN, :],
                            idxs_ap=idx_t[:],
                            num_idxs=wp, num_idxs_reg=wregs[wp], elem_size=D,
                            single_packet=False, queue_num=qn % 4)
                        nc.vector.tensor_add(out=accs[e][:, go:go + gw, :],
                                             in0=accs[e][:, go:go + gw, :], in1=msg[:])
                    qn += 1
                    col += cb
                for e in range(3):
                    flush(e, cur_r[e])

                # ---- dense stage: feature-major, transpose-free
                if skip_dense:
                    continue
                for h in range(2):
                    rows0 = h * HR
                    aggf = []
                    for e in range(3):
                        af = gpool.tile([P, HR], bf16, tag=f"agf{e}",
                                        name=f"agf_{l}_{h}_{e}")
                        nc.sync.dma_start(out=af[:],
                                          in_=masters[l][e][rows0:rows0 + HR, :],
                                          transpose=True)
                        rf = rpool.tile([D, HR], bf16, tag="rcf",
                                        name=f"rcf_{l}_{h}_{e}")
                        nc.sync.dma_start(out=rf[:],
                                          in_=recipf[e, :, rows0:rows0 + HR])
                        nc.vector.tensor_mul(out=af[0:D, :], in0=af[0:D, :], in1=rf[:])
                        aggf.append(af)

                    nblk = (HG + BL - 1) // BL
                    for b in range(nblk):
                        gw = min(BL, HG - b * BL)
                        c0 = b * BL * P           # col offset within half
                        cw = gw * P
                        gcol = rows0 + c0          # global node col offset
                        # t_dst chunks (feature-major, from DRAM)
                        tfd = [xf[0], xf[1]] if l == 0 else [tf1[0], tf1[1]]
                        tfa = dpool.tile([D, BL * P], bf16, tag="tfa", name=f"tfa_{l}_{h}_{b}")
                        tfb = dpool.tile([D, BL * P], bf16, tag="tfb", name=f"tfb_{l}_{h}_{b}")
                        nc.sync.dma_start(out=tfa[:, 0:cw], in_=tfd[0][:, gcol:gcol + cw])
                        nc.sync.dma_start(out=tfb[:, 0:cw], in_=tfd[1][:, gcol:gcol + cw])

                        tns = [dpool.tile([P, BL, D], f32, tag=f"tn{t}",
                                          name=f"tn{t}_{l}_{h}_{b}") for t in range(2)]
                        for gi in range(gw):
                            lc = c0 + gi * P
                            # type A node-major: aggBA@W + aggAA@W + tfa@Wr
                            pA = psA.tile([P, D], f32, tag="pa", name=f"pA_{l}_{h}_{b}_{gi}")
                            nc.tensor.matmul(out=pA[:], lhsT=aggf[1][0:D, lc:lc + P],
                                             rhs=wn_t[:, l * 3 + 1, :], start=True, stop=False)
                            nc.tensor.matmul(out=pA[:], lhsT=aggf[2][0:D, lc:lc + P],
                                             rhs=wn_t[:, l * 3 + 2, :], start=False, stop=False)
                            nc.tensor.matmul(out=pA[:], lhsT=tfa[:, gi * P:(gi + 1) * P],
                                             rhs=wr_t[:, l * 2 + 0, :], start=False, stop=True)
                            # type B node-major: aggAB@W + tfb@Wr
                            pB = psB.tile([P, D], f32, tag="pb", name=f"pB_{l}_{h}_{b}_{gi}")
                            nc.tensor.matmul(out=pB[:], lhsT=aggf[0][0:D, lc:lc + P],
                                             rhs=wn_t[:, l * 3 + 0, :], start=True, stop=False)
                            nc.tensor.matmul(out=pB[:], lhsT=tfb[:, gi * P:(gi + 1) * P],
                                             rhs=wr_t[:, l * 2 + 1, :], start=False, stop=True)
                            for t, ps in ((0, pA), (1, pB)):
                                nc.vector.tensor_add(out=tns[t][:, gi, :], in0=ps[:],
                                                     in1=brep_t[:, l * 2 + t, :])
                                if last:
                                    nc.scalar.activation(
                                        tns[t][:, gi, :], tns[t][:, gi, :],
                                        mybir.ActivationFunctionType.Tanh)
                        brearr = "(g p) d -> p g d"
                        if not last:
                            for t in range(2):
                                nc.sync.dma_start(
                                    out=stg[t][gcol:gcol + cw, :].rearrange(brearr, p=P),
                                    in_=tns[t][:, 0:gw, :])
                        else:
                            # fused gates: out = f*c + i*tanh_t
                            for t in range(2):
                                tname = "AB"[t]
                                ct = mpool.tile([P, WCAP_G, D], f32, tag="msg",
                                                name=f"ct{t}_{h}_{b}")
                                it = mpool.tile([P, WCAP_G, D], f32, tag="msg",
                                                name=f"it{t}_{h}_{b}")
                                ftl = mpool.tile([P, WCAP_G, D], f32, tag="msg",
                                                 name=f"ft{t}_{h}_{b}")
                                nc.sync.dma_start(
                                    out=ct[:, 0:gw, :],
                                    in_=gates_in["c" + tname][gcol:gcol + cw, :].rearrange(brearr, p=P))
                                nc.sync.dma_start(
                                    out=it[:, 0:gw, :],
                                    in_=gates_in["i" + tname][gcol:gcol + cw, :].rearrange(brearr, p=P))
                                nc.sync.dma_start(
                                    out=ftl[:, 0:gw, :],
                                    in_=gates_in["f" + tname][gcol:gcol + cw, :].rearrange(brearr, p=P))
                                nc.vector.tensor_mul(out=ftl[:, 0:gw, :], in0=ftl[:, 0:gw, :],
                                                     in1=ct[:, 0:gw, :])
                                nc.vector.tensor_mul(out=tns[t][:, 0:gw, :], in0=tns[t][:, 0:gw, :],
                                                     in1=it[:, 0:gw, :])
                                nc.vector.tensor_add(out=tns[t][:, 0:gw, :], in0=tns[t][:, 0:gw, :],
                                                     in1=ftl[:, 0:gw, :])
                                nc.sync.dma_start(
                                    out=outs[t][gcol:gcol + cw, :].rearrange(brearr, p=P),
                                    in_=tns[t][:, 0:gw, :])
                        # feature-major next-layer t (layer 0 only)
                        if not last:
                            for t, agl, tfx in ((0, (1, 2), tfa), (1, (0,), tfb)):
                                pF = (psFA if t == 0 else psFB).tile(
                                    [D, BL * P], f32, tag="pf", name=f"pF_{l}_{h}_{b}_{t}")
                                first = True
                                for e in agl:
                                    nc.tensor.matmul(out=pF[:, 0:cw],
                                                     lhsT=wn_t[:, l * 3 + e, :],
                                                     rhs=aggf[e][0:D, c0:c0 + cw],
                                                     start=first, stop=False)
                                    first = False
                                nc.tensor.matmul(out=pF[:, 0:cw],
                                                 lhsT=wr_t[:, l * 2 + t, :],
                                                 rhs=tfx[:, 0:cw],
                                                 start=False, stop=True)
                                tfo = dpool.tile([D, BL * P], bf16, tag=f"tfo{t}",
                                                 name=f"tfo_{l}_{h}_{b}_{t}")
                                nc.scalar.activation(
                                    tfo[:, 0:cw], pF[:, 0:cw],
                                    mybir.ActivationFunctionType.Identity,
                                    bias=blf_t[:, t:t + 1])
                                nc.sync.dma_start(out=tf1[t][:, gcol:gcol + cw],
                                                  in_=tfo[:, 0:cw])

                if not last:
                    for t in range(2):
                        if PAD > SH:
                            nc.sync.dma_start(out=stg[t][SH:PAD, :],
                                              in_=zero_small[0:PAD - SH, :])
                        if skip_cc:
                            nc.sync.dma_start(out=tab1[t][0:PAD, :], in_=stg[t][:])
                        else:
                            nc.gpsimd.collective_compute(
                                "AllGather", mybir.AluOpType.bypass,
                                replica_groups=[list(range(NCO))],
                                ins=[stg[t][:]], outs=[tab1[t][:]])

    # Align SWDGE queue_num with Tile's DMASW semaphore lane assignment:
    # each DMASW sem must only ever be updated from one SWDGE queue, and
    # Tile assigns lanes round-robin over the scheduled order. queue = lane%4.
    import re as _re
    qcount = {}
    for _ins in list(nc.inst_map.values()):
        if isinstance(_ins, (mybir.InstDMAGatherAnt, mybir.InstDMAScatterAddAnt)):
            _si = _ins.sync_info
            for _u in (_si.on_update or []):
                _m = _re.match(r"DMASW(\d+)", getattr(_u, "ant_name", "") or "")
                if _m:
                    _ins.queue_num = int(_m.group(1)) % 4
                    qcount[_ins.queue_num] = qcount.get(_ins.queue_num, 0) + 1
                    break
    nc._swdge_queue_dist = qcount

    nc.compile()
    return nc


# ---------------------------------------------------------------- host wrapper

def make_in_maps(cfg, inputs, percore):
    import ml_dtypes
    bf16 = ml_dtypes.bfloat16
    cfg = cfg_derived(cfg)
    NCO, SH, PAD, TROWS = cfg["n_cores"], cfg["shard"], cfg["pad"], cfg["trows"]

    def pad_rows(a):
        out = np.zeros((PAD, D), np.float32)
        out[:SH] = a
        return out

    # full x in table layout
    tabx = {}
    for t, xn in (("A", "x_A"), ("B", "x_B")):
        tb = np.zeros((TROWS, D), np.float32)
        x = np.asarray(inputs[xn], np.float32)
        for c in range(NCO):
            tb[PAD * c:PAD * c + SH] = x[SH * c:SH * (c + 1)]
        tabx[t] = tb

    Wl = np.asarray(inputs["Wl"], np.float32)
    Wr = np.asarray(inputs["Wr"], np.float32)
    bl = np.asarray(inputs["bl"], np.float32)
    WxA = np.asarray(inputs["Wx_A"], np.float32)
    WxB = np.asarray(inputs["Wx_B"], np.float32)
    biasA = np.asarray(inputs["bias_A"], np.float32)
    biasB = np.asarray(inputs["bias_B"], np.float32)
    Wx = [WxA, WxB]

    # wnode[l, e] = lhs-folded (Wl[l,e] @ Wx_src)^T for l==0 else Wl[l,e]^T
    wnode = np.zeros((L, 3, D, D), np.float32)
    for l in range(L):
        for e, (_, sT, dT) in enumerate(ETS):
            w = Wl[l, e] @ Wx[sT] if l == 0 else Wl[l, e]
            wnode[l, e] = w.T
    # wrn[l, t]: dst-type folded Wr sums
    wrn = np.zeros((L, 2, D, D), np.float32)
    for l in range(L):
        wrA = Wr[l, 1] + Wr[l, 2]
        wrB = Wr[l, 0]
        if l == 0:
            wrA = wrA @ WxA
            wrB = wrB @ WxB
        wrn[l, 0] = wrA.T
        wrn[l, 1] = wrB.T

    # biases: layer bl sums per dst type (+ final per-type bias at last layer)
    bl_t = np.zeros((L, 2, D), np.float32)
    for l in range(L):
        bl_t[l, 0] = bl[l, 1] + bl[l, 2]
        bl_t[l, 1] = bl[l, 0]
    bl_t[L - 1, 0] += biasA
    bl_t[L - 1, 1] += biasB
    biasrep = np.broadcast_to(bl_t[:, :, None, :], (L, 2, P, D)).copy()
    blf = np.ascontiguousarray(bl_t[0].T)  # [D, 2] (layer-0 feat-major bias)

    mz = np.zeros((PAD, P), bf16)

    in_maps = []
    for c in range(NCO):
        sl = slice(SH * c, SH * (c + 1))
        deg = percore[c]["deg"]  # [3, PAD] int32
        recipf = np.zeros((3, D, PAD), np.float32)
        recipf[:, :, :] = (1.0 / np.maximum(deg, 1.0))[:, None, :]
        xf = {}
        for t, xn in (("A", "x_A"), ("B", "x_B")):
            a = np.zeros((D, PAD), np.float32)
            a[:, :SH] = np.asarray(inputs[xn], np.float32)[sl].T
            xf[t] = a.astype(bf16)
        m = {
            "tabxA": tabx["A"], "tabxB": tabx["B"],
            "xfA": xf["A"], "xfB": xf["B"],
            "wnode": wnode.astype(bf16), "wrn": wrn.astype(bf16),
            "biasrep": biasrep, "blf": blf,
            "recipf": recipf.astype(bf16),
            "gidx": percore[c]["gidx"],
            "sidx": percore[c]["sidx"],
        }
        for l in range(L):
            for e in range(3):
                m[f"m{e}_{l}"] = mz
        for t in "AB":
            for nmm in "cif":
                m[f"{nmm}{t}"] = pad_rows(np.asarray(inputs[f"{nmm}_{t}"])[sl])
        in_maps.append(m)
    return in_maps


_BUILT = {}


def kernel(**inputs):
    from concourse.bass_utils import run_bass_kernel_spmd

    cfg0 = full_cfg()
    edges = {"AB": np.asarray(inputs["edge_AB"]),
             "BA": np.asarray(inputs["edge_BA"]),
             "AA": np.asarray(inputs["edge_AA"])}
    cfg, schedule, percore, cbtot = host_prep(cfg0, edges)

    key = (cbtot, tuple(schedule))
    if key not in _BUILT:
        _BUILT.clear()
        _BUILT[key] = build(cfg0, schedule, cbtot)
    nc = _BUILT[key]

    in_maps = make_in_maps(cfg0, inputs, percore)
    r = run_bass_kernel_spmd(nc, in_maps, core_ids=list(range(cfg["n_cores"])))

    SH = cfg["shard"]
    out_A = np.concatenate([r.results[c]["outA"][:SH] for c in range(cfg["n_cores"])], axis=0)
    out_B = np.concatenate([r.results[c]["outB"][:SH] for c in range(cfg["n_cores"])], axis=0)
    return (out_A, out_B)


# revision 32
# speedup vs baseline: 1.0880x; 1.0880x over previous
"""Trainium2 Bass kernel for nn_CellGate (hetero GNN message passing + LSTM-style gate).

Strategy (8-core SPMD, dst-sharded), v2:
- Each core owns a contiguous 12,500-node shard of both node types (A and B).
- Segment-mean aggregation per edge type via `dma_gather` slot passes (as v1),
  but with small pass widths (<=24 groups), deep idx/msg pools and round-robin
  SWDGE queues so all four Q7 descriptor-generation pairs run concurrently.
- Per-(et,range) flush: one DVE cast f32->bf16 of the slot accumulator, then 4
  `dma_scatter_add`s (bf16, 256B rows) into a host-zeroed DRAM master.
- Dense stage is transpose-free: masters are read back with HWDGE DMA-transpose
  into feature-major bf16 [64, nodes] tiles, recip (mean) applied there once,
  and per-128-node-chunk matmuls consume the same feature-major tile as lhsT
  (node-major output) and as rhs (feature-major output for the next layer's
  W_r term). Biases ride in via partition-replicated adds / activation bias.
- Weights (incl. the Wx input-projection folds for layer 0) are folded on host
  and shipped bf16.
- One AllGather per node type rebuilds the full f32 gather table between
  layers; gates are elementwise on DVE at the end.
"""

import numpy as np

import concourse.bass as bass
import concourse.bacc as bacc
import concourse.mybir as mybir
import concourse.tile as tile

P = 128
D = 64

# edge types: (name, src_type, dst_type)
ETS = [("AB", 0, 1), ("BA", 1, 0), ("AA", 0, 0)]
L = 2

WCAP_G = 24              # max gather pass width in groups of 128
WSET_G = list(range(1, 26))   # width register values (multiples of 128)


def full_cfg():
    return dict(n_cores=8, shard=12500, G=98, n_ranges=4, spr=2)


def cfg_derived(cfg):
    c = dict(cfg)
    c["pad"] = P * c["G"]
    c["rwin"] = c["spr"] * c["pad"]
    c["trows"] = c["n_cores"] * c["pad"]
    c["nnodes"] = c["n_cores"] * c["shard"]
    c["wset"] = [g * P for g in WSET_G]
    # scatter chunk group ranges (4 chunks per flush)
    sch = []
    g0 = 0
    while g0 < c["G"]:
        g1 = min(g0 + 25, c["G"])
        sch.append((g0, g1))
        g0 = g1
    c["scatter_chunks"] = sch
    return c


def roundw(w, wset):
    for v in wset:
        if v >= w:
            return v
    return wset[-1]


# ---------------------------------------------------------------- host prep

def host_prep(cfg, edges):
    """edges: dict name -> [2, E] int32 (src, dst global).

    Returns: schedule (static, shared): list of passes (et_i, r, k, go, wp)
    and per-core arrays:
      gidx[core]: int16 [128, CBTOT]  (packed gather indices, 8-replicated)
      sidx[core]: int16 [3, n_ranges, 128, pad//16]
      deg[core]:  int32 [3, pad]   (total in-degree per et)
    """
    cfg = cfg_derived(cfg)
    NCO, SH, PAD, G = cfg["n_cores"], cfg["shard"], cfg["pad"], cfg["G"]
    NR, RWIN, WSET = cfg["n_ranges"], cfg["rwin"], cfg["wset"]

    percore = [dict(sidx=np.zeros((3, NR, 128, PAD // 16), np.int16),
                    deg=np.zeros((3, PAD), np.int32)) for _ in range(NCO)]

    all_counts = {}   # (et_i, r, k) -> max over cores of count
    maxk = {}         # (et_i, r) -> K
    core_data = {}    # (core, et_i, r) -> (theta, by_k list)
    for et_i, (etn, sT, dT) in enumerate(ETS):
        src, dst = edges[etn][0].astype(np.int64), edges[etn][1].astype(np.int64)
        srow = PAD * (src // SH) + (src - SH * (src // SH))  # global table row
        for c in range(NCO):
            m = (dst // SH) == c
            s_r, d_l = srow[m], dst[m] - c * SH
            percore[c]["deg"][et_i] = np.bincount(d_l, minlength=PAD)[:PAD]
            for r in range(NR):
                rm = (s_r // RWIN) == r
                sl, dl = s_r[rm] - r * RWIN, d_l[rm]
                deg_r = np.bincount(dl, minlength=PAD)[:PAD]
                theta = np.argsort(-deg_r, kind="stable")
                slot_of = np.empty(PAD, np.int64)
                slot_of[theta] = np.arange(PAD)
                K = int(deg_r.max()) if deg_r.size else 0
                maxk[(et_i, r)] = max(maxk.get((et_i, r), 0), K)
                order = np.argsort(dl, kind="stable")
                dls, sls = dl[order], sl[order]
                if dls.size:
                    starts = np.r_[0, np.nonzero(np.diff(dls))[0] + 1]
                    rank = np.arange(dls.size) - np.repeat(
                        starts, np.diff(np.r_[starts, dls.size]))
                else:
                    rank = np.zeros(0, np.int64)
                by_k = []
                for k in range(K):
                    km = rank == k
                    dk, sk = dls[km], sls[km]
                    all_counts[(et_i, r, k)] = max(
                        all_counts.get((et_i, r, k), 0), dk.size)
                    by_k.append((slot_of[dk], sk))
                core_data[(c, et_i, r)] = (theta, by_k)
                w = theta.astype(np.int16).reshape(PAD // 16, 16).T
                percore[c]["sidx"][et_i, r] = np.tile(w, (8, 1))

    # static schedule, pieces of <= WCAP_G groups, round-robin across ets
    schedule = []
    for et_i in range(3):
        for r in range(NR):
            for k in range(maxk.get((et_i, r), 0)):
                w = ((max(all_counts.get((et_i, r, k), 1), 1) + P - 1) // P) * P
                go = 0
                while go * P < w:
                    gw = min(WCAP_G, w // P - go)
                    wp = roundw(gw * P, WSET)
                    schedule.append((et_i, r, k, go, wp))
                    go += wp // P
    # stagger range order per et so flushes don't cluster across ets
    def rord(e, r):
        return (r - e) % NR
    per_et = [[] for _ in range(3)]
    for t in sorted(schedule, key=lambda t: (t[0], rord(t[0], t[1]), t[2], t[3])):
        per_et[t[0]].append(t)
    schedule = []
    i = [0, 0, 0]
    while any(i[e] < len(per_et[e]) for e in range(3)):
        for e in range(3):
            if i[e] < len(per_et[e]):
                schedule.append(per_et[e][i[e]])
                i[e] += 1

    # packed gather idx per core
    ZLOC = SH  # zero-row local index within each range window
    for c in range(NCO):
        full_arr = {}
        for (et_i, r, k, go, wp) in schedule:
            key = (c, et_i, r, k)
            if key not in full_arr:
                theta, by_k = core_data[(c, et_i, r)]
                arr = np.full(PAD, ZLOC, np.int16)
                if k < len(by_k):
                    slots, srcs = by_k[k]
                    arr[slots] = srcs.astype(np.int16)
                full_arr[key] = arr
        parts = []
        for (et_i, r, k, go, wp) in schedule:
            arr = np.full(wp, ZLOC, np.int16)
            seg = full_arr[(c, et_i, r, k)][go * P:go * P + wp]
            arr[:seg.size] = seg
            wrapped = arr.reshape(wp // 16, 16).T
            parts.append(np.tile(wrapped, (8, 1)))
        percore[c]["gidx"] = np.concatenate(parts, axis=1)
    cbtot = percore[0]["gidx"].shape[1]
    return cfg, schedule, percore, cbtot


# ---------------------------------------------------------------- builder

def build(cfg, schedule, cbtot, skip_gather=False, skip_dense=False,
          skip_cc=False, skip_scatter=False):
    cfg = cfg_derived(cfg)
    NCO, SH, PAD, G = cfg["n_cores"], cfg["shard"], cfg["pad"], cfg["G"]
    NR, RWIN, TROWS, WSET = cfg["n_ranges"], cfg["rwin"], cfg["trows"], cfg["wset"]
    f32 = mybir.dt.float32
    bf16 = mybir.dt.bfloat16
    i16 = mybir.dt.int16
    HG = 49                    # groups per dense half
    HR = HG * P                # rows per dense half
    BL = 4                     # dense groups per block
    GCHUNK = 20                # gates groups per chunk

    nc = bacc.Bacc(None, target_bir_lowering=False, debug=False,
                   num_swdge_queues=4, num_devices=NCO)

    # ---------------- inputs
    tabx = [nc.declare_dram_parameter(f"tabx{t}", [TROWS, D], f32, isOutput=False)
            for t in "AB"]
    xf = [nc.declare_dram_parameter(f"xf{t}", [D, PAD], bf16, isOutput=False)
          for t in "AB"]
    gates_in = {}
    for t in "AB":
        for nmm in "cif":
            gates_in[nmm + t] = nc.declare_dram_parameter(
                f"{nmm}{t}", [PAD, D], f32, isOutput=False)
    wcomb = nc.declare_dram_parameter("wcomb", [L, 2, P, D], bf16, isOutput=False)
    wrn = nc.declare_dram_parameter("wrn", [L, 2, D, D], bf16, isOutput=False)
    biasrep = nc.declare_dram_parameter("biasrep", [L, 2, P, D], f32, isOutput=False)
    blf = nc.declare_dram_parameter("blf", [D, 2], f32, isOutput=False)
    recipf = nc.declare_dram_parameter("recipf", [2, P, PAD], bf16, isOutput=False)
    # per dst type: type A holds BA sums in cols 0:64 and AA sums in 64:128
    masters = [[nc.declare_dram_parameter(f"m{t}_{l}", [PAD, P], bf16,
                                          isOutput=False)
                for t in "AB"] for l in range(L)]
    gidx = nc.declare_dram_parameter("gidx", [128, cbtot], i16, isOutput=False)
    sidx = nc.declare_dram_parameter("sidx", [3, NR, 128, PAD // 16], i16,
                                     isOutput=False)
    outs = [nc.declare_dram_parameter(f"out{t}", [PAD, D], f32, isOutput=True)
            for t in "AB"]

    # ---------------- DRAM internals
    stg = [nc.dram_tensor(f"stg{t}", [PAD, D], f32) for t in "AB"]
    tf1 = [nc.dram_tensor(f"tf1{t}", [D, PAD], bf16) for t in "AB"]
    tab_space = "Shared" if NCO > 4 else "Local"
    tab1 = [nc.dram_tensor(f"tab1{t}", [TROWS, D], f32, addr_space=tab_space)
            for t in "AB"]

    # ---------------- width registers (before TileContext)
    wregs = {}
    for w in WSET:
        r = nc.alloc_register(mybir.EngineType.Pool, f"w{w}")
        nc.gpsimd.reg_mov(r, w)
        wregs[w] = r

    rearr = "(p g) d -> p g d"

    with tile.TileContext(nc) as tc:
        with tc.tile_pool(name="const", bufs=1) as cpool, \
             tc.tile_pool(name="accp", bufs=2) as apool, \
             tc.tile_pool(name="idxp", bufs=16) as ipool, \
             tc.tile_pool(name="sidxp", bufs=2) as spool, \
             tc.tile_pool(name="msgp", bufs=8) as mpool, \
             tc.tile_pool(name="aggfp", bufs=1) as gpool, \
             tc.tile_pool(name="recfp", bufs=2) as rpool, \
             tc.tile_pool(name="densep", bufs=3) as dpool, \
             tc.tile_pool(name="psA", bufs=2, space="PSUM") as psA, \
             tc.tile_pool(name="psB", bufs=2, space="PSUM") as psB, \
             tc.tile_pool(name="psFA", bufs=2, space="PSUM") as psFA, \
             tc.tile_pool(name="psFB", bufs=2, space="PSUM") as psFB:

            # ---- constants
            wc_t = cpool.tile([P, L * 2, D], bf16)
            nc.sync.dma_start(out=wc_t[:], in_=wcomb[:].rearrange("l t a b -> a (l t) b"))
            wr_t = cpool.tile([D, L * 2, D], bf16)
            nc.sync.dma_start(out=wr_t[:], in_=wrn[:].rearrange("l t a b -> a (l t) b"))
            brep_t = cpool.tile([P, L * 2, D], f32)
            nc.sync.dma_start(out=brep_t[:], in_=biasrep[:].rearrange("l t p d -> p (l t) d"))
            blf_t = cpool.tile([D, 2], f32)
            nc.sync.dma_start(out=blf_t[:], in_=blf[:])

            zero_small = cpool.tile([P, D], f32)
            nc.vector.memset(zero_small[:], 0.0)

            # ============ per layer ============
            for l in range(L):
                tabs = [tabx[0], tabx[1]] if l == 0 else [tab1[0], tab1[1]]
                last = (l == L - 1)

                accs = [apool.tile([P, G, D], bf16, tag=f"acc{e}", name=f"acc_{l}_{e}")
                        for e in range(3)]
                cur_r = [None, None, None]
                for (e, r, k, go, wp) in schedule:
                    if cur_r[e] is None:
                        cur_r[e] = r
                for e in range(3):
                    nc.vector.memset(accs[e][:], 0.0)

                def flush(e, r, accs=accs, l=l):
                    # scatter-add the bf16 slot accumulator into the master
                    sx = spool.tile([P, PAD // 16], i16, tag="sx",
                                    name=f"sx_{l}_{e}_{r}")
                    nc.sync.dma_start(out=sx[:], in_=sidx[e, r])
                    if skip_scatter:
                        return
                    # et -> (dst-type master, column half): AB->B[0:], BA->A[0:], AA->A[64:]
                    mt, cof = ((1, 0), (0, 0), (0, D))[e]
                    for (g0, g1) in cfg["scatter_chunks"]:
                        w = (g1 - g0) * P
                        nc.gpsimd.dma_scatter_add(
                            masters[l][mt][:, cof:cof + D], accs[e][:, g0:g1, :],
                            sx[:, g0 * 8:g1 * 8], w, wregs[w], D,
                            elem_step=P,
                            single_packet=False, queue_num=0)

                col = 0
                qn = 0
                for (e, r, k, go, wp) in schedule:
                    if r != cur_r[e]:
                        flush(e, cur_r[e])
                        cur_r[e] = r
                        accs[e] = apool.tile([P, G, D], bf16, tag=f"acc{e}",
                                             name=f"acc_{l}_{e}_r{r}")
                        nc.vector.memset(accs[e][:], 0.0)
                    cb = wp // 16
                    gw = wp // P
                    idx_t = ipool.tile([P, cb], i16, tag="gi", name=f"gi_{l}_{qn}")
                    nc.sync.dma_start(out=idx_t[:], in_=gidx[:, col:col + cb])
                    msg = mpool.tile([P, gw, D], f32, tag="msg", name=f"msg_{l}_{qn}")
                    sT = ETS[e][1]
                    if not skip_gather:
                        nc.gpsimd.dma_gather(
                            out_ap=msg[:],
                            in_ap=tabs[sT][r * RWIN:(r + 1) * RWIN, :],
                            idxs_ap=idx_t[:],
                            num_idxs=wp, num_idxs_reg=wregs[wp], elem_size=D,
                            single_packet=False, queue_num=qn % 4)
                        nc.vector.tensor_add(out=accs[e][:, go:go + gw, :],
                                             in0=accs[e][:, go:go + gw, :], in1=msg[:])
                    qn += 1
                    col += cb
                for e in range(3):
                    flush(e, cur_r[e])

                # ---- dense stage: feature-major, transpose-free
                if skip_dense:
                    continue
                for h in range(2):
                    rows0 = h * HR
                    aggf = []
                    for t in range(2):
                        af = gpool.tile([P, HR], bf16, tag=f"agf{t}",
                                        name=f"agf_{l}_{h}_{t}")
                        nc.sync.dma_start(out=af[:],
                                          in_=masters[l][t][rows0:rows0 + HR, :],
                                          transpose=True)
                        rf = rpool.tile([P, HR], bf16, tag="rcf",
                                        name=f"rcf_{l}_{h}_{t}")
                        nc.sync.dma_start(out=rf[:],
                                          in_=recipf[t, :, rows0:rows0 + HR])
                        nc.vector.tensor_mul(out=af[:], in0=af[:], in1=rf[:])
                        aggf.append(af)

                    nblk = (HG + BL - 1) // BL
                    for b in range(nblk):
                        gw = min(BL, HG - b * BL)
                        c0 = b * BL * P           # col offset within half
                        cw = gw * P
                        gcol = rows0 + c0          # global node col offset
                        # t_dst chunks (feature-major, from DRAM)
                        tfd = [xf[0], xf[1]] if l == 0 else [tf1[0], tf1[1]]
                        tfa = dpool.tile([D, BL * P], bf16, tag="tfa", name=f"tfa_{l}_{h}_{b}")
                        tfb = dpool.tile([D, BL * P], bf16, tag="tfb", name=f"tfb_{l}_{h}_{b}")
                        nc.sync.dma_start(out=tfa[:, 0:cw], in_=tfd[0][:, gcol:gcol + cw])
                        nc.sync.dma_start(out=tfb[:, 0:cw], in_=tfd[1][:, gcol:gcol + cw])

                        tns = [dpool.tile([P, BL, D], f32, tag=f"tn{t}",
                                          name=f"tn{t}_{l}_{h}_{b}") for t in range(2)]
                        for gi in range(gw):
                            lc = c0 + gi * P
                            # type A node-major: [aggBA;aggAA]@Wcomb + tfa@Wr
                            pA = psA.tile([P, D], f32, tag="pa", name=f"pA_{l}_{h}_{b}_{gi}")
                            nc.tensor.matmul(out=pA[:], lhsT=aggf[0][:, lc:lc + P],
                                             rhs=wc_t[:, l * 2 + 0, :], start=True, stop=False)
                            nc.tensor.matmul(out=pA[:], lhsT=tfa[:, gi * P:(gi + 1) * P],
                                             rhs=wr_t[:, l * 2 + 0, :], start=False, stop=True)
                            # type B node-major: aggAB@Wcomb + tfb@Wr
                            pB = psB.tile([P, D], f32, tag="pb", name=f"pB_{l}_{h}_{b}_{gi}")
                            nc.tensor.matmul(out=pB[:], lhsT=aggf[1][:, lc:lc + P],
                                             rhs=wc_t[:, l * 2 + 1, :], start=True, stop=False)
                            nc.tensor.matmul(out=pB[:], lhsT=tfb[:, gi * P:(gi + 1) * P],
                                             rhs=wr_t[:, l * 2 + 1, :], start=False, stop=True)
                            for t, ps in ((0, pA), (1, pB)):
                                nc.vector.tensor_add(out=tns[t][:, gi, :], in0=ps[:],
                                                     in1=brep_t[:, l * 2 + t, :])
                                if last:
                                    nc.scalar.activation(
                                        tns[t][:, gi, :], tns[t][:, gi, :],
                                        mybir.ActivationFunctionType.Tanh)
                        brearr = "(g p) d -> p g d"
                        if not last:
                            for t in range(2):
                                nc.sync.dma_start(
                                    out=stg[t][gcol:gcol + cw, :].rearrange(brearr, p=P),
                                    in_=tns[t][:, 0:gw, :])
                        else:
                            # fused gates: out = f*c + i*tanh_t
                            for t in range(2):
                                tname = "AB"[t]
                                ct = mpool.tile([P, WCAP_G, D], f32, tag="msg",
                                                name=f"ct{t}_{h}_{b}")
                                it = mpool.tile([P, WCAP_G, D], f32, tag="msg",
                                                name=f"it{t}_{h}_{b}")
                                ftl = mpool.tile([P, WCAP_G, D], f32, tag="msg",
                                                 name=f"ft{t}_{h}_{b}")
                                nc.sync.dma_start(
                                    out=ct[:, 0:gw, :],
                                    in_=gates_in["c" + tname][gcol:gcol + cw, :].rearrange(brearr, p=P))
                                nc.sync.dma_start(
                                    out=it[:, 0:gw, :],
                                    in_=gates_in["i" + tname][gcol:gcol + cw, :].rearrange(brearr, p=P))
                                nc.sync.dma_start(
                                    out=ftl[:, 0:gw, :],
                                    in_=gates_in["f" + tname][gcol:gcol + cw, :].rearrange(brearr, p=P))
                                nc.vector.tensor_mul(out=ftl[:, 0:gw, :], in0=ftl[:, 0:gw, :],
                                                     in1=ct[:, 0:gw, :])
                                nc.vector.tensor_mul(out=tns[t][:, 0:gw, :], in0=tns[t][:, 0:gw, :],
                                                     in1=it[:, 0:gw, :])
                                nc.vector.tensor_add(out=tns[t][:, 0:gw, :], in0=tns[t][:, 0:gw, :],
                                                     in1=ftl[:, 0:gw, :])
                                nc.sync.dma_start(
                                    out=outs[t][gcol:gcol + cw, :].rearrange(brearr, p=P),
                                    in_=tns[t][:, 0:gw, :])
                        # feature-major next-layer t (layer 0 only)
                        if not last:
                            for t, tfx in ((0, tfa), (1, tfb)):
                                pF = (psFA if t == 0 else psFB).tile(
                                    [D, BL * P], f32, tag="pf", name=f"pF_{l}_{h}_{b}_{t}")
                                nc.tensor.matmul(out=pF[:, 0:cw],
                                                 lhsT=wc_t[:, l * 2 + t, :],
                                                 rhs=aggf[t][:, c0:c0 + cw],
                                                 start=True, stop=False)
                                nc.tensor.matmul(out=pF[:, 0:cw],
                                                 lhsT=wr_t[:, l * 2 + t, :],
                                                 rhs=tfx[:, 0:cw],
                                                 start=False, stop=True)
                                tfo = dpool.tile([D, BL * P], bf16, tag=f"tfo{t}",
                                                 name=f"tfo_{l}_{h}_{b}_{t}")
                                nc.scalar.activation(
                                    tfo[:, 0:cw], pF[:, 0:cw],
                                    mybir.ActivationFunctionType.Identity,
                                    bias=blf_t[:, t:t + 1])
                                nc.sync.dma_start(out=tf1[t][:, gcol:gcol + cw],
                                                  in_=tfo[:, 0:cw])

                if not last:
                    for t in range(2):
                        if PAD > SH:
                            nc.sync.dma_start(out=stg[t][SH:PAD, :],
                                              in_=zero_small[0:PAD - SH, :])
                        if skip_cc:
                            nc.sync.dma_start(out=tab1[t][0:PAD, :], in_=stg[t][:])
                        else:
                            nc.gpsimd.collective_compute(
                                "AllGather", mybir.AluOpType.bypass,
                                replica_groups=[list(range(NCO))],
                                ins=[stg[t][:]], outs=[tab1[t][:]])

    # Align SWDGE queue_num with Tile's DMASW semaphore lane assignment:
    # each DMASW sem must only ever be updated from one SWDGE queue, and
    # Tile assigns lanes round-robin over the scheduled order. queue = lane%4.
    import re as _re
    qcount = {}
    for _ins in list(nc.inst_map.values()):
        if isinstance(_ins, (mybir.InstDMAGatherAnt, mybir.InstDMAScatterAddAnt)):
            _si = _ins.sync_info
            for _u in (_si.on_update or []):
                _m = _re.match(r"DMASW(\d+)", getattr(_u, "ant_name", "") or "")
                if _m:
                    _ins.queue_num = int(_m.group(1)) % 4
                    qcount[_ins.queue_num] = qcount.get(_ins.queue_num, 0) + 1
                    break
    nc._swdge_queue_dist = qcount

    nc.compile()
    return nc


# ---------------------------------------------------------------- host wrapper

def make_in_maps(cfg, inputs, percore):
    import ml_dtypes
    bf16 = ml_dtypes.bfloat16
    cfg = cfg_derived(cfg)
    NCO, SH, PAD, TROWS = cfg["n_cores"], cfg["shard"], cfg["pad"], cfg["trows"]

    def pad_rows(a):
        out = np.zeros((PAD, D), np.float32)
        out[:SH] = a
        return out

    # full x in table layout
    tabx = {}
    for t, xn in (("A", "x_A"), ("B", "x_B")):
        tb = np.zeros((TROWS, D), np.float32)
        x = np.asarray(inputs[xn], np.float32)
        for c in range(NCO):
            tb[PAD * c:PAD * c + SH] = x[SH * c:SH * (c + 1)]
        tabx[t] = tb

    Wl = np.asarray(inputs["Wl"], np.float32)
    Wr = np.asarray(inputs["Wr"], np.float32)
    bl = np.asarray(inputs["bl"], np.float32)
    WxA = np.asarray(inputs["Wx_A"], np.float32)
    WxB = np.asarray(inputs["Wx_B"], np.float32)
    biasA = np.asarray(inputs["bias_A"], np.float32)
    biasB = np.asarray(inputs["bias_B"], np.float32)
    Wx = [WxA, WxB]

    # wnode[l, e] = lhs-folded (Wl[l,e] @ Wx_src)^T for l==0 else Wl[l,e]^T
    wnode = np.zeros((L, 3, D, D), np.float32)
    for l in range(L):
        for e, (_, sT, dT) in enumerate(ETS):
            w = Wl[l, e] @ Wx[sT] if l == 0 else Wl[l, e]
            wnode[l, e] = w.T
    # combined per-dst-type weights: A stacks [BA; AA] (K=128), B stacks [AB; 0]
    wcomb = np.zeros((L, 2, P, D), np.float32)
    for l in range(L):
        wcomb[l, 0, 0:D] = wnode[l, 1]
        wcomb[l, 0, D:P] = wnode[l, 2]
        wcomb[l, 1, 0:D] = wnode[l, 0]
    # wrn[l, t]: dst-type folded Wr sums
    wrn = np.zeros((L, 2, D, D), np.float32)
    for l in range(L):
        wrA = Wr[l, 1] + Wr[l, 2]
        wrB = Wr[l, 0]
        if l == 0:
            wrA = wrA @ WxA
            wrB = wrB @ WxB
        wrn[l, 0] = wrA.T
        wrn[l, 1] = wrB.T

    # biases: layer bl sums per dst type (+ final per-type bias at last layer)
    bl_t = np.zeros((L, 2, D), np.float32)
    for l in range(L):
        bl_t[l, 0] = bl[l, 1] + bl[l, 2]
        bl_t[l, 1] = bl[l, 0]
    bl_t[L - 1, 0] += biasA
    bl_t[L - 1, 1] += biasB
    biasrep = np.broadcast_to(bl_t[:, :, None, :], (L, 2, P, D)).copy()
    blf = np.ascontiguousarray(bl_t[0].T)  # [D, 2] (layer-0 feat-major bias)

    mz = np.zeros((PAD, P), bf16)

    in_maps = []
    for c in range(NCO):
        sl = slice(SH * c, SH * (c + 1))
        deg = percore[c]["deg"]  # [3, PAD] int32
        rec = 1.0 / np.maximum(deg, 1.0)       # [3, PAD]
        recipf = np.zeros((2, P, PAD), np.float32)
        recipf[0, 0:D] = rec[1][None, :]       # A: BA in rows 0:64
        recipf[0, D:P] = rec[2][None, :]       # A: AA in rows 64:128
        recipf[1, 0:D] = rec[0][None, :]       # B: AB in rows 0:64
        xf = {}
        for t, xn in (("A", "x_A"), ("B", "x_B")):
            a = np.zeros((D, PAD), np.float32)
            a[:, :SH] = np.asarray(inputs[xn], np.float32)[sl].T
            xf[t] = a.astype(bf16)
        m = {
            "tabxA": tabx["A"], "tabxB": tabx["B"],
            "xfA": xf["A"], "xfB": xf["B"],
            "wcomb": wcomb.astype(bf16), "wrn": wrn.astype(bf16),
            "biasrep": biasrep, "blf": blf,
            "recipf": recipf.astype(bf16),
            "gidx": percore[c]["gidx"],
            "sidx": percore[c]["sidx"],
        }
        for l in range(L):
            for t in "AB":
                m[f"m{t}_{l}"] = mz
        for t in "AB":
            for nmm in "cif":
                m[f"{nmm}{t}"] = pad_rows(np.asarray(inputs[f"{nmm}_{t}"])[sl])
        in_maps.append(m)
    return in_maps


_BUILT = {}


def kernel(**inputs):
    from concourse.bass_utils import run_bass_kernel_spmd

    cfg0 = full_cfg()
    edges = {"AB": np.asarray(inputs["edge_AB"]),
             "BA": np.asarray(inputs["edge_BA"]),
             "AA": np.asarray(inputs["edge_AA"])}
    cfg, schedule, percore, cbtot = host_prep(cfg0, edges)

    key = (cbtot, tuple(schedule))
    if key not in _BUILT:
        _BUILT.clear()
        _BUILT[key] = build(cfg0, schedule, cbtot)
    nc = _BUILT[key]

    in_maps = make_in_maps(cfg0, inputs, percore)
    r = run_bass_kernel_spmd(nc, in_maps, core_ids=list(range(cfg["n_cores"])))

    SH = cfg["shard"]
    out_A = np.concatenate([r.results[c]["outA"][:SH] for c in range(cfg["n_cores"])], axis=0)
    out_B = np.concatenate([r.results[c]["outB"][:SH] for c in range(cfg["n_cores"])], axis=0)
    return (out_A, out_B)
